# revision 53
# baseline (speedup 1.0000x reference)
"""Causal self-attention (GQA + RoPE) Trainium2 Bass kernel, 8 NeuronCores.

Sharding: 2-way data parallel over batch x 4-way tensor parallel over heads.
Core c handles batch c//4 and query heads [4*(c%4), 4*(c%4)+4) plus the one
KV head g = c%4 that serves them (n_kv_heads=4 -> no KV replication).
Each core computes a partial [S, D] output (its heads' slice of the out
projection); the host sums the 4 partials per batch.

Device layouts are transposed ("feature-major"): projections produce qT/kT/vT
[dim, tokens]; attention scores are computed as S^T = kT.T @ qT.  RoPE is
handled by de-interleaving the q/k weight rows on the host so the rotation
pairs become (p, p+64) partition pairs.

Token chunks are processed 0..3 in DMA-arrival order (causal attention for
chunk c only needs K/V of chunks <= c), with x/wq split into sub-DMAs
spread over the sync+gpsimd queues (each DMA queue sustains only ~140GB/s)
so the PE starts ~12us in and never starves.  All TensorEngine operands are
fp16 (fp32 PSUM accumulation).  Softmax denominators are accumulated on the
Vector engine and reduced over partitions with one ones-vector matmul per
(chunk, head) — keeping the per-block partition-sum matmuls off the PE.
exp() uses a -2 bias (softmax shift invariance) so fp16 e-values stay in
range.  Outputs are written fp16 on two alternating queues; the host sums
the 4 partials per batch in fp32.
"""

import sys

if "/opt/trn_rl_repo" not in sys.path:
    sys.path.insert(0, "/opt/trn_rl_repo")

import math

import numpy as np

D_MODEL = 2048
N_HEADS = 16
N_KV_HEADS = 4
ROPE_THETA = 10000.0
B, S = 2, 2048
DK = D_MODEL // N_HEADS          # 128
NCORES = 8
NEG = -1e30

_COMPILED = None
_TRACE = False                   # test.py flips this for profiling runs
_LAST_RESULT = None              # BassKernelResults of the last run


def _build():
    import concourse.bacc as bacc
    import concourse.tile as tile
    from concourse import bass_isa, mybir

    f32 = mybir.dt.float32
    f16 = mybir.dt.float16

    nc = bacc.Bacc("TRN2", debug=False, target_bir_lowering=False)

    def inp(name, shape, dt=f16):
        return nc.declare_dram_parameter(name, list(shape), dt, isOutput=False).ap()

    x_d = inp("x", [128, 4, 16, 512])          # [part, chunk, db, tok]
    wq_d = inp("wq", [128, 4, 16, 128])        # [part, m, db, mcol]
    wkv_d = inp("wkv", [128, 2, 16, 128])      # [part, k/v, db, col]
    wc_d = inp("wc", [128, 4, 2048])
    tab_d = inp("tab", [128, 4, 2, 512])       # [part, chunk, cos/sin, tok]
    dmask_d = inp("dmask", [128, 128], f32)
    out_d = nc.declare_dram_parameter("out", [S, D_MODEL], f16, isOutput=True).ap()

    EXP = mybir.ActivationFunctionType.Exp

    with tile.TileContext(nc) as tc:
        with (
            tc.tile_pool(name="consts", bufs=1) as consts,
            tc.tile_pool(name="qpool", bufs=4) as qpool,
            tc.tile_pool(name="vch", bufs=2) as vchp,
            tc.tile_pool(name="tmp", bufs=2) as tmpp,
            tc.tile_pool(name="epool", bufs=8) as epool,
            tc.tile_pool(name="accp", bufs=2) as accp,
            tc.tile_pool(name="rsum", bufs=2) as rsp,
            tc.tile_pool(name="otp", bufs=4) as otp,
            tc.tile_pool(name="osb", bufs=6) as osbp,
            tc.tile_pool(name="psum_st", bufs=3, space="PSUM") as psum_st,
            tc.tile_pool(name="psum_ot", bufs=1, space="PSUM") as psum_otp,
            tc.tile_pool(name="psum_nrm", bufs=1, space="PSUM") as psum_nrm,
            tc.tile_pool(name="psum_gen", bufs=3, space="PSUM") as psum_gen,
        ):
            # ---- constants / weights ----
            wq_sb = consts.tile([128, 4, 16, 128], f16, tag="wq")
            wkv_sb = consts.tile([128, 2, 16, 128], f16, tag="wkv")
            wc_sb = consts.tile([128, 4, 2048], f16, tag="wc")
            tab_sb = consts.tile([128, 4, 2, 512], f16, tag="tab")
            dmask_sb = consts.tile([128, 128], f32, tag="dmask")
            onescol_sb = consts.tile([128, 1], f16, tag="onescol")
            onesrow_sb = consts.tile([1, 128], f16, tag="onesrow")
            kTr_sb = consts.tile([128, S], f16, tag="kTr")
            v_sb = consts.tile([128, 16, 128], f16, tag="V")
            xT = consts.tile([128, 4, 16, 512], f16, tag="xT")
            ebias_sb = consts.tile([128, 1], f32, tag="ebias")
            nc.gpsimd.memset(ebias_sb, -2.0)
            nc.gpsimd.memset(onescol_sb, 1.0)
            nc.gpsimd.memset(onesrow_sb, 1.0)

            # DMA plan.  The three queues (sync/gpsimd/scalar) each move
            # ~140GB/s after a slow ramp, so the startup-critical tensors
            # (x chunk 0, wkv, wq m0) are spread across all three in
            # first-use order; everything else streams behind.
            nc.sync.dma_start(out=xT[:, 0, 0:4, :], in_=x_d[:, 0, 0:4, :])
            nc.gpsimd.dma_start(out=xT[:, 0, 8:12, :], in_=x_d[:, 0, 8:12, :])
            nc.sync.dma_start(out=xT[:, 0, 4:8, :], in_=x_d[:, 0, 4:8, :])
            nc.gpsimd.dma_start(out=xT[:, 0, 12:16, :], in_=x_d[:, 0, 12:16, :])
            nc.scalar.dma_start(out=wkv_sb[:, 1], in_=wkv_d[:, 1])   # V half
            nc.scalar.dma_start(out=wkv_sb[:, 0], in_=wkv_d[:, 0])   # K half
            nc.scalar.dma_start(out=tab_sb[:, 0], in_=tab_d[:, 0])
            nc.scalar.dma_start(out=wq_sb[:, 0], in_=wq_d[:, 0])
            nc.gpsimd.dma_start(out=wq_sb[:, 2], in_=wq_d[:, 2])
            nc.sync.dma_start(out=dmask_sb, in_=dmask_d)
            nc.scalar.dma_start(out=wq_sb[:, 1], in_=wq_d[:, 1])
            nc.gpsimd.dma_start(out=wq_sb[:, 3], in_=wq_d[:, 3])
            nc.sync.dma_start(out=xT[:, 1], in_=x_d[:, 1])
            for c in range(1, 4):
                nc.scalar.dma_start(out=tab_sb[:, c], in_=tab_d[:, c])
            nc.scalar.dma_start(out=wc_sb, in_=wc_d)
            nc.sync.dma_start(out=xT[:, 2], in_=x_d[:, 2])
            nc.gpsimd.dma_start(out=xT[:, 3], in_=x_d[:, 3])

            def rope(dst, src, c):
                """dst[128,512] (f16 SBUF) <- rotate(src[128,512] f32 PSUM).

                Row p<64 holds the even (te) element of pair p, row p+64 the
                odd (to): dst_lo = te*cos - to*sin; dst_hi = to*cos + te*sin.
                """
                cs = tab_sb[:, c, 0, :]
                sn = tab_sb[:, c, 1, :]
                t = tmpp.tile([128, 512], f32, tag="ropesin")
                t2 = tmpp.tile([128, 512], f32, tag="ropecos")
                nc.vector.tensor_mul(t[0:64, :], src[64:128, :], sn[0:64, :])
                nc.vector.tensor_mul(t[64:128, :], src[0:64, :], sn[64:128, :])
                nc.vector.tensor_mul(t2, src, cs)
                nc.vector.tensor_add(dst, t2, t)

            qTrs = {}
            psum = psum_gen       # proj + out-proj share one 3-bank pool

            # accumulation order follows x chunk 0's quarter-DMA arrival
            DB_ORDER = [0, 1, 2, 3, 8, 9, 10, 11, 4, 5, 6, 7, 12, 13, 14, 15]

            def emit_qproj(c, m):
                if c in qTrs:
                    qTr = qTrs[c]
                else:
                    qTr = qpool.tile([128, 4, 512], f16, tag="qTr")
                    qTrs[c] = qTr
                pq = psum.tile([128, 512], f32, tag="mm512")
                for i, db in enumerate(DB_ORDER):
                    nc.tensor.matmul(
                        pq,
                        lhsT=wq_sb[:, m, db, :],
                        rhs=xT[:, c, db, :],
                        start=(i == 0),
                        stop=(i == 15),
                    )
                rope(qTr[:, m, :], pq, c)

            def emit_kproj(c):
                pk = psum.tile([128, 512], f32, tag="mm512")
                for i, db in enumerate(DB_ORDER):
                    nc.tensor.matmul(
                        pk,
                        lhsT=wkv_sb[:, 0, db, :],
                        rhs=xT[:, c, db, :],
                        start=(i == 0),
                        stop=(i == 15),
                    )
                rope(kTr_sb[:, c * 512:(c + 1) * 512], pk, c)

            def emit_vproj(c):
                pv = psum.tile([128, 512], f32, tag="mm512")
                for i, db in enumerate(DB_ORDER):
                    nc.tensor.matmul(
                        pv,
                        lhsT=wkv_sb[:, 1, db, :],
                        rhs=xT[:, c, db, :],
                        start=(i == 0),
                        stop=(i == 15),
                    )
                vch = vchp.tile([128, 512], f16, tag="vch")
                nc.scalar.copy(out=vch, in_=pv)
                for rr in range(4):
                    nc.sync.dma_start_transpose(
                        out=v_sb[:, 4 * c + rr, :],
                        in_=vch[:, rr * 128:(rr + 1) * 128],
                    )

            _dmaq = [0]

            def emit_outproj(tq0, otc, final=False):
                for tb in range(4):
                    row = tq0 + tb * 128
                    for oc in range(4):
                        po = psum_gen.tile([128, 512], f32, tag="mm512")
                        for h in range(4):
                            nc.tensor.matmul(
                                po,
                                lhsT=otc[:, h, tb * 128:(tb + 1) * 128],
                                rhs=wc_sb[:, h, oc * 512:(oc + 1) * 512],
                                start=(h == 0),
                                stop=(h == 3),
                            )
                        osb = osbp.tile([128, 512], f16, tag="osb")
                        # exp is done by the final out-projection, so the
                        # idle Scalar engine takes half its PSUM drains
                        if final and oc % 2 == 1:
                            nc.scalar.copy(out=osb, in_=po)
                        else:
                            nc.vector.tensor_copy(out=osb, in_=po)
                        q = (nc.gpsimd, nc.sync)[_dmaq[0] % 2]
                        _dmaq[0] += 1
                        q.dma_start(
                            out=out_d[row:row + 128, oc * 512:(oc + 1) * 512],
                            in_=osb,
                        )

            def emit_attn(c):
                """Attention for token chunk c, all 4 heads -> otc tile."""
                nkb = 4 * c + 4
                qTr = qTrs[c]
                otc = otp.tile([128, 4, 512], f16, tag="OT")
                for h in range(4):
                    psum_ot = psum_otp.tile([128, 512], f32, tag="ot")
                    acc = accp.tile([128, 512], f16, tag="acc")

                    def st_mm(kb):
                        """Score matmul (+ causal mask) for one key block."""
                        rr = kb - 4 * c  # >= 0 on the diagonal chunk group
                        col0 = 0 if rr < 0 else 128 * rr
                        pst = psum_st.tile([128, 512], f32, tag="st")
                        nc.tensor.matmul(
                            pst[:, col0:512],
                            lhsT=kTr_sb[:, kb * 128:(kb + 1) * 128],
                            rhs=qTr[:, h, col0:512],
                            start=True,
                            stop=True,
                        )
                        if rr >= 0:
                            nc.vector.tensor_add(
                                pst[:, col0:col0 + 128],
                                pst[:, col0:col0 + 128],
                                dmask_sb,
                            )
                        return pst, col0

                    # software-pipelined two ahead: the PE never head-blocks
                    # on the activation engine's exp
                    pending = [st_mm(0), st_mm(1)] if nkb > 1 else [st_mm(0)]
                    for kb in range(nkb):
                        pst, col0 = pending.pop(0)
                        if kb + 2 < nkb:
                            pending.append(st_mm(kb + 2))
                        e = epool.tile([128, 512], f16, tag="E")
                        nc.scalar.activation(
                            out=e[:, col0:512], in_=pst[:, col0:512], func=EXP,
                            bias=ebias_sb,
                        )
                        # denominator accumulation on DVE (keeps the
                        # partition-sum matmuls off the Tensor engine)
                        if kb == 0:
                            nc.vector.tensor_copy(out=acc, in_=e)
                        else:
                            nc.vector.tensor_add(
                                acc[:, col0:512], acc[:, col0:512],
                                e[:, col0:512],
                            )
                        nc.tensor.matmul(
                            psum_ot[:, col0:512],
                            lhsT=v_sb[:, kb, :],
                            rhs=e[:, col0:512],
                            start=(kb == 0),
                            stop=(kb == nkb - 1),
                        )
                    psum_sum = psum_nrm.tile([128, 512], f32, tag="nrm")
                    nc.tensor.matmul(
                        psum_sum[0:1, :], lhsT=onescol_sb, rhs=acc,
                        start=True, stop=True,
                    )
                    rsum = rsp.tile([1, 512], f32, tag="rsum")
                    rsumb = rsp.tile([1, 512], f16, tag="rsumb")
                    nc.vector.reciprocal_approx_fast(out=rsum, in_=psum_sum[0:1, :])
                    nc.vector.tensor_copy(out=rsumb, in_=rsum)
                    pb = psum_nrm.tile([128, 512], f32, tag="nrm")
                    nc.tensor.matmul(
                        pb, lhsT=onesrow_sb, rhs=rsumb, start=True, stop=True
                    )
                    # PSUM has a single DVE read port: stage psum_ot to SBUF
                    # on the Scalar engine, then scale by pb on DVE.
                    otr = rsp.tile([128, 512], f16, tag="otraw")
                    nc.scalar.copy(out=otr, in_=psum_ot)
                    nc.vector.tensor_mul(otc[:, h, :], otr, pb)
                return otc

            # chunk-streamed: project chunk c as its x arrives, run its
            # attention (needs only chunks <= c), defer its out-projection
            # one attention so those matmuls fill the next attention's
            # stalls.  Attention order 0,1,3,2 keeps out-proj filler
            # available through the final attention + normalization chain.
            for c in range(4):
                emit_vproj(c)
                emit_kproj(c)
                for m in ((0, 2, 1, 3) if c == 0 else range(4)):
                    emit_qproj(c, m)
            # attention biggest-chunk first: the big attentions overlap the
            # tail of the projection stream, and each out-projection
            # (deferred one attention) fills the next attention's
            # exp-latency stalls; the kernel ends with the smallest
            # attention + a full out-proj block hiding the final drain
            outproj_queue = []
            for c in (3, 2, 1, 0):
                otc = emit_attn(c)
                outproj_queue.append((c * 512, otc))
                if len(outproj_queue) >= 2:
                    emit_outproj(*outproj_queue.pop(0))
            emit_outproj(*outproj_queue.pop(0), final=True)

    nc.compile()
    return nc


def _host_prep(x, Wq, Wkv, Wc):
    """Shard + relayout the full inputs into the 8 per-core input dicts."""
    f16 = np.float16
    dk, H, KV = DK, N_HEADS, N_KV_HEADS
    x = np.asarray(x, np.float32)
    Wq = np.asarray(Wq, np.float32)
    Wkv = np.asarray(Wkv, np.float32)
    Wc = np.asarray(Wc, np.float32)

    p = np.concatenate([np.arange(0, dk, 2), np.arange(1, dk, 2)])
    perm_q = np.concatenate([h * dk + p for h in range(H)])
    Wq_p = (Wq / math.sqrt(dk))[perm_q]
    perm_k = np.concatenate([g * dk + p for g in range(KV)])
    Wk_p = Wkv[:KV * dk][perm_k]
    Wv = Wkv[KV * dk:]

    pairs = np.arange(dk // 2, dtype=np.float64)
    freqs = 1.0 / (ROPE_THETA ** (2.0 * pairs / dk))
    ang = np.arange(S, dtype=np.float64)[:, None] * freqs[None, :]
    cos_t = np.cos(ang).astype(np.float32).T  # [64, S]
    sin_t = np.sin(ang).astype(np.float32).T
    c2 = np.concatenate([cos_t, cos_t], 0).reshape(128, 4, 512)
    ss = np.concatenate([-sin_t, sin_t], 0).reshape(128, 4, 512)
    tab = np.ascontiguousarray(np.stack([c2, ss], 2)).astype(f16)

    jj = np.arange(128)[None, :]
    pp = np.arange(128)[:, None]
    dmask = np.where(pp <= jj, 0.0, NEG).astype(np.float32)

    maps = []
    for core in range(NCORES):
        b, g = core // 4, core % 4
        wq_l = np.ascontiguousarray(
            Wq_p[512 * g:512 * g + 512].T.reshape(16, 128, 4, 128)
            .transpose(1, 2, 0, 3)
        ).astype(f16)
        wkv_sl = np.stack(
            [Wk_p[g * dk:(g + 1) * dk].T, Wv[g * dk:(g + 1) * dk].T], 0
        )  # [2, 2048, 128]
        wkv_l = np.ascontiguousarray(
            wkv_sl.reshape(2, 16, 128, 128).transpose(2, 0, 1, 3)
        ).astype(f16)
        wc_l = np.ascontiguousarray(
            Wc[:, 512 * g:512 * g + 512].T.reshape(4, 128, 2048).transpose(1, 0, 2)
        ).astype(f16)
        xt_l = np.ascontiguousarray(
            x[b].T.reshape(16, 128, 4, 512).transpose(1, 2, 0, 3)
        ).astype(f16)
        maps.append(dict(
            x=xt_l, wq=wq_l, wkv=wkv_l, wc=wc_l,
            tab=tab, dmask=dmask,
        ))
    return maps


def kernel(x, Wq, Wkv, Wc):
    global _COMPILED, _LAST_RESULT
    from concourse.bass_utils import run_bass_kernel_spmd

    if _COMPILED is None:
        _COMPILED = _build()
    in_maps = _host_prep(x, Wq, Wkv, Wc)
    res = run_bass_kernel_spmd(
        _COMPILED, in_maps, core_ids=list(range(NCORES)), trace=_TRACE
    )
    _LAST_RESULT = res
    outs = [res.results[i]["out"].astype(np.float32) for i in range(NCORES)]
    full = np.stack(
        [outs[0] + outs[1] + outs[2] + outs[3],
         outs[4] + outs[5] + outs[6] + outs[7]], 0
    )
    return full


# revision 54
# speedup vs baseline: 1.0422x; 1.0422x over previous
"""Causal self-attention (GQA + RoPE) Trainium2 Bass kernel, 8 NeuronCores.

Sharding: 2-way data parallel over batch x 4-way tensor parallel over heads.
Core c handles batch c//4 and query heads [4*(c%4), 4*(c%4)+4) plus the one
KV head g = c%4 that serves them (n_kv_heads=4 -> no KV replication).
Each core computes a partial [S, D] output (its heads' slice of the out
projection); the host sums the 4 partials per batch.

Device layouts are transposed ("feature-major"): projections produce qT/kT/vT
[dim, tokens]; attention scores are computed as S^T = kT.T @ qT.  RoPE is
handled by de-interleaving the q/k weight rows on the host so the rotation
pairs become (p, p+64) partition pairs.

Token chunks are processed 0..3 in DMA-arrival order (causal attention for
chunk c only needs K/V of chunks <= c), with x/wq split into sub-DMAs
spread over the sync+gpsimd queues (each DMA queue sustains only ~140GB/s)
so the PE starts ~12us in and never starves.  All TensorEngine operands are
fp16 (fp32 PSUM accumulation).  Softmax denominators are accumulated on the
Vector engine and reduced over partitions with one ones-vector matmul per
(chunk, head) — keeping the per-block partition-sum matmuls off the PE.
exp() uses a -2 bias (softmax shift invariance) so fp16 e-values stay in
range.  Outputs are written fp16 on two alternating queues; the host sums
the 4 partials per batch in fp32.
"""

import sys

if "/opt/trn_rl_repo" not in sys.path:
    sys.path.insert(0, "/opt/trn_rl_repo")

import math

import numpy as np

D_MODEL = 2048
N_HEADS = 16
N_KV_HEADS = 4
ROPE_THETA = 10000.0
B, S = 2, 2048
DK = D_MODEL // N_HEADS          # 128
NCORES = 8
NEG = -1e30

_COMPILED = None
_TRACE = False                   # test.py flips this for profiling runs
_LAST_RESULT = None              # BassKernelResults of the last run


def _build():
    import concourse.bacc as bacc
    import concourse.tile as tile
    from concourse import bass_isa, mybir

    f32 = mybir.dt.float32
    f16 = mybir.dt.float16

    nc = bacc.Bacc("TRN2", debug=False, target_bir_lowering=False)

    def inp(name, shape, dt=f16):
        return nc.declare_dram_parameter(name, list(shape), dt, isOutput=False).ap()

    x_d = inp("x", [128, 4, 16, 512])          # [part, chunk, db, tok]
    wq_d = inp("wq", [128, 4, 16, 128])        # [part, m, db, mcol]
    wkv_d = inp("wkv", [128, 2, 16, 128])      # [part, k/v, db, col]
    wc_d = inp("wc", [128, 4, 2048])
    tab_d = inp("tab", [128, 4, 2, 512])       # [part, chunk, cos/sin, tok]
    dmask_d = inp("dmask", [128, 128], f32)
    out_d = nc.declare_dram_parameter("out", [S, D_MODEL], f16, isOutput=True).ap()

    EXP = mybir.ActivationFunctionType.Exp

    with tile.TileContext(nc) as tc:
        with (
            tc.tile_pool(name="consts", bufs=1) as consts,
            tc.tile_pool(name="qpool", bufs=4) as qpool,
            tc.tile_pool(name="vch", bufs=2) as vchp,
            tc.tile_pool(name="tmp", bufs=2) as tmpp,
            tc.tile_pool(name="epool", bufs=8) as epool,
            tc.tile_pool(name="accp", bufs=2) as accp,
            tc.tile_pool(name="rsum", bufs=2) as rsp,
            tc.tile_pool(name="otp", bufs=3) as otp,
            tc.tile_pool(name="osb", bufs=4) as osbp,
            tc.tile_pool(name="psum_st", bufs=3, space="PSUM") as psum_st,
            tc.tile_pool(name="psum_ot", bufs=1, space="PSUM") as psum_otp,
            tc.tile_pool(name="psum_nrm", bufs=1, space="PSUM") as psum_nrm,
            tc.tile_pool(name="psum_gen", bufs=3, space="PSUM") as psum_gen,
        ):
            # ---- constants / weights ----
            wq_sb = consts.tile([128, 4, 16, 128], f16, tag="wq")
            wkv_sb = consts.tile([128, 2, 16, 128], f16, tag="wkv")
            wc_sb = consts.tile([128, 4, 2048], f16, tag="wc")
            tab_sb = consts.tile([128, 4, 2, 512], f16, tag="tab")
            dmask_sb = consts.tile([128, 128], f32, tag="dmask")
            onescol_sb = consts.tile([128, 1], f16, tag="onescol")
            onesrow_sb = consts.tile([1, 128], f16, tag="onesrow")
            kTr_sb = consts.tile([128, S], f16, tag="kTr")
            v_sb = consts.tile([128, 16, 128], f16, tag="V")
            xT = consts.tile([128, 4, 16, 512], f16, tag="xT")
            ebias_sb = consts.tile([128, 1], f32, tag="ebias")
            nc.gpsimd.memset(ebias_sb, -2.0)
            nc.gpsimd.memset(onescol_sb, 1.0)
            nc.gpsimd.memset(onesrow_sb, 1.0)

            # DMA plan.  The three queues (sync/gpsimd/scalar) each move
            # ~140GB/s after a slow ramp, so the startup-critical tensors
            # (x chunk 0, wkv, wq m0) are spread across all three in
            # first-use order; everything else streams behind.
            nc.sync.dma_start(out=xT[:, 0, 0:8, :], in_=x_d[:, 0, 0:8, :])
            nc.gpsimd.dma_start(out=xT[:, 0, 8:16, :], in_=x_d[:, 0, 8:16, :])
            nc.scalar.dma_start(out=wkv_sb[:, 1], in_=wkv_d[:, 1])   # V half
            nc.scalar.dma_start(out=wkv_sb[:, 0], in_=wkv_d[:, 0])   # K half
            nc.scalar.dma_start(out=tab_sb[:, 0], in_=tab_d[:, 0])
            nc.sync.dma_start(out=dmask_sb, in_=dmask_d)
            for m in range(4):
                nc.gpsimd.dma_start(out=wq_sb[:, m], in_=wq_d[:, m])
            nc.sync.dma_start(out=xT[:, 1], in_=x_d[:, 1])
            for c in range(1, 4):
                nc.scalar.dma_start(out=tab_sb[:, c], in_=tab_d[:, c])
            nc.scalar.dma_start(out=wc_sb, in_=wc_d)
            nc.sync.dma_start(out=xT[:, 2], in_=x_d[:, 2])
            nc.gpsimd.dma_start(out=xT[:, 3], in_=x_d[:, 3])

            def rope(dst, src, c):
                """dst[128,512] (f16 SBUF) <- rotate(src[128,512] f32 PSUM).

                Row p<64 holds the even (te) element of pair p, row p+64 the
                odd (to): dst_lo = te*cos - to*sin; dst_hi = to*cos + te*sin.
                """
                cs = tab_sb[:, c, 0, :]
                sn = tab_sb[:, c, 1, :]
                t = tmpp.tile([128, 512], f32, tag="ropesin")
                t2 = tmpp.tile([128, 512], f32, tag="ropecos")
                nc.vector.tensor_mul(t[0:64, :], src[64:128, :], sn[0:64, :])
                nc.vector.tensor_mul(t[64:128, :], src[0:64, :], sn[64:128, :])
                nc.vector.tensor_mul(t2, src, cs)
                nc.vector.tensor_add(dst, t2, t)

            qTrs = {}
            psum = psum_gen       # proj + out-proj share one 3-bank pool

            def emit_qproj(c, m):
                if c in qTrs:
                    qTr = qTrs[c]
                else:
                    qTr = qpool.tile([128, 4, 512], f16, tag="qTr")
                    qTrs[c] = qTr
                pq = psum.tile([128, 512], f32, tag="mm512")
                for db in range(16):
                    nc.tensor.matmul(
                        pq,
                        lhsT=wq_sb[:, m, db, :],
                        rhs=xT[:, c, db, :],
                        start=(db == 0),
                        stop=(db == 15),
                    )
                rope(qTr[:, m, :], pq, c)

            def emit_kproj(c):
                pk = psum.tile([128, 512], f32, tag="mm512")
                for db in range(16):
                    nc.tensor.matmul(
                        pk,
                        lhsT=wkv_sb[:, 0, db, :],
                        rhs=xT[:, c, db, :],
                        start=(db == 0),
                        stop=(db == 15),
                    )
                rope(kTr_sb[:, c * 512:(c + 1) * 512], pk, c)

            def emit_vproj(c):
                pv = psum.tile([128, 512], f32, tag="mm512")
                for db in range(16):
                    nc.tensor.matmul(
                        pv,
                        lhsT=wkv_sb[:, 1, db, :],
                        rhs=xT[:, c, db, :],
                        start=(db == 0),
                        stop=(db == 15),
                    )
                vch = vchp.tile([128, 512], f16, tag="vch")
                nc.scalar.copy(out=vch, in_=pv)
                for rr in range(4):
                    nc.sync.dma_start_transpose(
                        out=v_sb[:, 4 * c + rr, :],
                        in_=vch[:, rr * 128:(rr + 1) * 128],
                    )

            _dmaq = [0]

            def emit_outproj(tq0, otc, final=False):
                for tb in range(4):
                    row = tq0 + tb * 128
                    for oc in range(4):
                        po = psum_gen.tile([128, 512], f32, tag="mm512")
                        for h in range(4):
                            nc.tensor.matmul(
                                po,
                                lhsT=otc[:, h, tb * 128:(tb + 1) * 128],
                                rhs=wc_sb[:, h, oc * 512:(oc + 1) * 512],
                                start=(h == 0),
                                stop=(h == 3),
                            )
                        osb = osbp.tile([128, 512], f16, tag="osb")
                        # exp is done by the final out-projection, so the
                        # idle Scalar engine takes half its PSUM drains
                        if final and oc % 2 == 1:
                            nc.scalar.copy(out=osb, in_=po)
                        else:
                            nc.vector.tensor_copy(out=osb, in_=po)
                        q = (nc.gpsimd, nc.sync)[_dmaq[0] % 2]
                        _dmaq[0] += 1
                        q.dma_start(
                            out=out_d[row:row + 128, oc * 512:(oc + 1) * 512],
                            in_=osb,
                        )

            def emit_attn(c):
                """Attention for token chunk c, all 4 heads -> otc tile."""
                nkb = 4 * c + 4
                qTr = qTrs[c]
                otc = otp.tile([128, 4, 512], f16, tag="OT")
                for h in range(4):
                    psum_ot = psum_otp.tile([128, 512], f32, tag="ot")
                    acc = accp.tile([128, 512], f16, tag="acc")

                    def st_mm(kb):
                        """Score matmul (+ causal mask) for one key block."""
                        rr = kb - 4 * c  # >= 0 on the diagonal chunk group
                        col0 = 0 if rr < 0 else 128 * rr
                        pst = psum_st.tile([128, 512], f32, tag="st")
                        nc.tensor.matmul(
                            pst[:, col0:512],
                            lhsT=kTr_sb[:, kb * 128:(kb + 1) * 128],
                            rhs=qTr[:, h, col0:512],
                            start=True,
                            stop=True,
                        )
                        if rr >= 0:
                            nc.vector.tensor_add(
                                pst[:, col0:col0 + 128],
                                pst[:, col0:col0 + 128],
                                dmask_sb,
                            )
                        return pst, col0

                    # software-pipelined two ahead: the PE never head-blocks
                    # on the activation engine's exp
                    pending = [st_mm(0), st_mm(1)] if nkb > 1 else [st_mm(0)]
                    for kb in range(nkb):
                        pst, col0 = pending.pop(0)
                        if kb + 2 < nkb:
                            pending.append(st_mm(kb + 2))
                        e = epool.tile([128, 512], f16, tag="E")
                        nc.scalar.activation(
                            out=e[:, col0:512], in_=pst[:, col0:512], func=EXP,
                            bias=ebias_sb,
                        )
                        # denominator accumulation on DVE (keeps the
                        # partition-sum matmuls off the Tensor engine)
                        if kb == 0:
                            nc.vector.tensor_copy(out=acc, in_=e)
                        else:
                            nc.vector.tensor_add(
                                acc[:, col0:512], acc[:, col0:512],
                                e[:, col0:512],
                            )
                        nc.tensor.matmul(
                            psum_ot[:, col0:512],
                            lhsT=v_sb[:, kb, :],
                            rhs=e[:, col0:512],
                            start=(kb == 0),
                            stop=(kb == nkb - 1),
                        )
                    psum_sum = psum_nrm.tile([128, 512], f32, tag="nrm")
                    nc.tensor.matmul(
                        psum_sum[0:1, :], lhsT=onescol_sb, rhs=acc,
                        start=True, stop=True,
                    )
                    rsum = rsp.tile([1, 512], f32, tag="rsum")
                    rsumb = rsp.tile([1, 512], f16, tag="rsumb")
                    nc.vector.reciprocal_approx_fast(out=rsum, in_=psum_sum[0:1, :])
                    nc.vector.tensor_copy(out=rsumb, in_=rsum)
                    pb = psum_nrm.tile([128, 512], f32, tag="nrm")
                    nc.tensor.matmul(
                        pb, lhsT=onesrow_sb, rhs=rsumb, start=True, stop=True
                    )
                    # PSUM has a single DVE read port: stage psum_ot to SBUF
                    # on the Scalar engine, then scale by pb on DVE.
                    otr = rsp.tile([128, 512], f16, tag="otraw")
                    nc.scalar.copy(out=otr, in_=psum_ot)
                    nc.vector.tensor_mul(otc[:, h, :], otr, pb)
                return otc

            # chunk-streamed: project chunk c as its x arrives, run its
            # attention (needs only chunks <= c), defer its out-projection
            # one attention so those matmuls fill the next attention's
            # stalls.  Attention order 0,1,3,2 keeps out-proj filler
            # available through the final attention + normalization chain.
            for c in range(4):
                emit_vproj(c)
                emit_kproj(c)
                for m in range(4):
                    emit_qproj(c, m)
            # attention biggest-chunk first: the big attentions overlap the
            # tail of the projection stream, and each out-projection
            # (deferred one attention) fills the next attention's
            # exp-latency stalls; the kernel ends with the smallest
            # attention + a full out-proj block hiding the final drain
            outproj_queue = []
            for c in (0, 1, 3, 2):
                otc = emit_attn(c)
                outproj_queue.append((c * 512, otc))
                if len(outproj_queue) >= 2:
                    emit_outproj(*outproj_queue.pop(0))
            emit_outproj(*outproj_queue.pop(0))

    nc.compile()
    return nc


def _host_prep(x, Wq, Wkv, Wc):
    """Shard + relayout the full inputs into the 8 per-core input dicts."""
    f16 = np.float16
    dk, H, KV = DK, N_HEADS, N_KV_HEADS
    x = np.asarray(x, np.float32)
    Wq = np.asarray(Wq, np.float32)
    Wkv = np.asarray(Wkv, np.float32)
    Wc = np.asarray(Wc, np.float32)

    p = np.concatenate([np.arange(0, dk, 2), np.arange(1, dk, 2)])
    perm_q = np.concatenate([h * dk + p for h in range(H)])
    Wq_p = (Wq / math.sqrt(dk))[perm_q]
    perm_k = np.concatenate([g * dk + p for g in range(KV)])
    Wk_p = Wkv[:KV * dk][perm_k]
    Wv = Wkv[KV * dk:]

    pairs = np.arange(dk // 2, dtype=np.float64)
    freqs = 1.0 / (ROPE_THETA ** (2.0 * pairs / dk))
    ang = np.arange(S, dtype=np.float64)[:, None] * freqs[None, :]
    cos_t = np.cos(ang).astype(np.float32).T  # [64, S]
    sin_t = np.sin(ang).astype(np.float32).T
    c2 = np.concatenate([cos_t, cos_t], 0).reshape(128, 4, 512)
    ss = np.concatenate([-sin_t, sin_t], 0).reshape(128, 4, 512)
    tab = np.ascontiguousarray(np.stack([c2, ss], 2)).astype(f16)

    jj = np.arange(128)[None, :]
    pp = np.arange(128)[:, None]
    dmask = np.where(pp <= jj, 0.0, NEG).astype(np.float32)

    maps = []
    for core in range(NCORES):
        b, g = core // 4, core % 4
        wq_l = np.ascontiguousarray(
            Wq_p[512 * g:512 * g + 512].T.reshape(16, 128, 4, 128)
            .transpose(1, 2, 0, 3)
        ).astype(f16)
        wkv_sl = np.stack(
            [Wk_p[g * dk:(g + 1) * dk].T, Wv[g * dk:(g + 1) * dk].T], 0
        )  # [2, 2048, 128]
        wkv_l = np.ascontiguousarray(
            wkv_sl.reshape(2, 16, 128, 128).transpose(2, 0, 1, 3)
        ).astype(f16)
        wc_l = np.ascontiguousarray(
            Wc[:, 512 * g:512 * g + 512].T.reshape(4, 128, 2048).transpose(1, 0, 2)
        ).astype(f16)
        xt_l = np.ascontiguousarray(
            x[b].T.reshape(16, 128, 4, 512).transpose(1, 2, 0, 3)
        ).astype(f16)
        maps.append(dict(
            x=xt_l, wq=wq_l, wkv=wkv_l, wc=wc_l,
            tab=tab, dmask=dmask,
        ))
    return maps


def kernel(x, Wq, Wkv, Wc):
    global _COMPILED, _LAST_RESULT
    from concourse.bass_utils import run_bass_kernel_spmd

    if _COMPILED is None:
        _COMPILED = _build()
    in_maps = _host_prep(x, Wq, Wkv, Wc)
    res = run_bass_kernel_spmd(
        _COMPILED, in_maps, core_ids=list(range(NCORES)), trace=_TRACE
    )
    _LAST_RESULT = res
    outs = [res.results[i]["out"].astype(np.float32) for i in range(NCORES)]
    full = np.stack(
        [outs[0] + outs[1] + outs[2] + outs[3],
         outs[4] + outs[5] + outs[6] + outs[7]], 0
    )
    return full
